# revision 14
# baseline (speedup 1.0000x reference)
"""CrossCosineEmbeddingLoss kernel for 8 trn2 NeuronCores.

loss = mean over all (i,j) of: 1 - cos(x_i, y_j) if i==j else relu(cos(x_i, y_j))

Identity:  total = sum_ij relu(sim_ij) + sum_i (1 - sim_ii - relu(sim_ii))
Sharding: rows of x across 8 cores (1024 rows each); y replicated: yt
(column-major, matmul stationary operand), yn (row-major, for 1/||y_j||),
yd (local 1024 rows, diagonal terms). All host-side reshapes are pure
layout permutations (no host arithmetic).

v16: fully decoupled cores — NO collective. Cross-core launch skew made
every core wait ~10-15us on the AllGather mesh, so each core now computes
all 8192 y-norms locally: Pool squares yn slabs (otherwise-idle engine),
DVE does segmented tensor_reduce [128,16,128]->[128,16] (one op per 16
tiles), recip+sqrt gives rny in [j%128, j/128] layout directly — no
transposes, no gather.

x / yd / yn are host-permuted so each partition's DMA data is contiguous
(128 big descriptors per tensor instead of 1024 x 512B).

Main: 64 j-tiles: 2 f32r matmuls -> [128,1024] fp32 PSUM -> fused
relu+accum split ACT (PSUM accum) / DVE (SBUF accum).
Host combines [128,2] partials from each core.
"""

import numpy as np

import concourse.bacc as bacc
import concourse.bass as bass
import concourse.tile as tile
from concourse import mybir
from concourse.bass_utils import run_bass_kernel_spmd
from concourse.masks import make_identity

N, D = 8192, 128
NCORES = 8
SH = N // NCORES          # 1024 rows of x per core
TX = SH // 128            # 8 x-tiles per core
TY = N // 128             # 64 y-tiles
RG = 4                    # segmented-reduce groups for ny^2
RT = TY // RG             # tiles per reduce group

f32 = mybir.dt.float32
f32r = mybir.dt.float32r
AF = mybir.ActivationFunctionType
ALU = mybir.AluOpType
AX = mybir.AxisListType

ACT_TILES = 38              # of 64 main tiles handled by ACT (rest DVE)


def _reduce_kind(t):
    lead = 2 * ACT_TILES - TY
    if t < lead:
        return "act"
    return "dve" if (t - lead) % 2 == 0 else "act"


_CACHE = {}


def _build():
    if "nc" in _CACHE:
        return _CACHE["nc"]
    nc = bacc.Bacc("TRN2", target_bir_lowering=False, debug=False,
                   num_devices=NCORES)
    xs_d = nc.dram_tensor("xs", [SH, D], f32, kind="ExternalInput")
    yd_d = nc.dram_tensor("yd", [SH, D], f32, kind="ExternalInput")
    yt_d = nc.dram_tensor("yt", [D, N], f32r, kind="ExternalInput")
    yn_d = nc.dram_tensor("yn", [N, D], f32, kind="ExternalInput")
    out_d = nc.dram_tensor("out", [128, 2], f32, kind="ExternalOutput")

    with tile.TileContext(nc) as tc:
        with (
            tc.tile_pool(name="singles", bufs=1) as singles,
            tc.tile_pool(name="scrD", bufs=2) as scrD,
        ):
            yT32 = singles.tile([128, TY, 128], f32r)   # [d, jt, j]
            ynat = singles.tile([128, TY, 128], f32)    # all y rows
            sqn = singles.tile([128, TY, 128], f32)     # yn squared
            ydnat = singles.tile([128, TX, 128], f32)   # local y rows
            xnat = singles.tile([128, TX, 128], f32)    # [i%128, it, d]
            xhat = singles.tile([128, TX, 128], f32)
            xhatT = singles.tile([128, TX, 128], f32r)  # [d, it, i]
            ident = singles.tile([128, 128], f32)
            nx2 = singles.tile([128, TX], f32)
            rnx = singles.tile([128, TX], f32)
            nyd2 = singles.tile([128, TX], f32)
            rnyd = singles.tile([128, TX], f32)
            ny2 = singles.tile([128, TY], f32)
            rny = singles.tile([128, TY], f32)
            R = singles.tile([128, TY], f32)
            d2 = singles.tile([128, TX], f32)
            t1x = singles.tile([128, TX], f32)
            sim_d = singles.tile([128, TX], f32)
            relu_d = singles.tile([128, TX], f32)
            outsb = singles.tile([128, 2], f32)
            warm = singles.tile([128, 8], f32)

            # preload the sqrt+relu activation table set early
            nc.gpsimd.memset(warm[:], 1.0)
            nc.scalar.sqrt(warm[:], warm[:])
            make_identity(nc, ident[:])
            nc.gpsimd.memset(outsb[:], 0.0)
            nc.gpsimd.memset(R[:], 0.0)

            # ---- DMA: x first (gates the main loop), yd, then yT chunks
            # interleaved with yn halves (yn only needed mid-kernel)
            nc.sync.dma_start(
                out=xnat[:],
                in_=xs_d[:].rearrange("(p t) d -> p t d", p=128))
            nc.sync.dma_start(
                out=ydnat[:],
                in_=yd_d[:].rearrange("(p t) d -> p t d", p=128))

            def yt_chunk(a, b):
                nc.sync.dma_start(
                    out=yT32[:, a:b, :],
                    in_=yt_d[:, 128 * a:128 * b]
                    .rearrange("p (t j) -> p t j", j=128))

            def yn_chunk(a, b):
                nc.sync.dma_start(
                    out=ynat[:, a:b, :],
                    in_=yn_d[:].rearrange("(p t) d -> p t d", p=128)[:, a:b, :])

            yt_chunk(0, 4)
            yt_chunk(4, 12)
            yt_chunk(12, 24)
            yn_chunk(0, 32)
            yt_chunk(24, 40)
            yn_chunk(32, TY)
            yt_chunk(40, TY)

            # ---- Pool: square yn slabs (feeds DVE segmented reduces)
            for s8 in range(TY // 8):
                sl = slice(8 * s8, 8 * (s8 + 1))
                nc.gpsimd.tensor_mul(sqn[:, sl, :], ynat[:, sl, :],
                                     ynat[:, sl, :])

            # ---- x prep: DVE sumsq+scale, ACT rsqrt-chain, PE transposes
            with tc.tile_pool(name="tpsum", bufs=1, space="PSUM") as tpsum:
                ptx = tpsum.tile([128, 1024], f32, tag="tp")
                xt_flat = xhatT[:].rearrange("p a b -> p (a b)")
                for t in range(TX):
                    nc.vector.scalar_tensor_tensor(
                        out=scrD.tile([128, 128], f32, tag='sd', name='sd')[:],
                        in0=xnat[:, t, :], scalar=1.0, in1=xnat[:, t, :],
                        op0=ALU.mult, op1=ALU.mult,
                        accum_out=nx2[:, t:t + 1])
                    nc.vector.reciprocal(t1x[:, t:t + 1], nx2[:, t:t + 1])
                    nc.scalar.sqrt(rnx[:, t:t + 1], t1x[:, t:t + 1])
                    nc.vector.tensor_scalar(
                        out=xhat[:, t, :], in0=xnat[:, t, :],
                        scalar1=rnx[:, t:t + 1], scalar2=None,
                        op0=ALU.mult, op1=ALU.bypass)
                    nc.tensor.transpose(ptx[:, 128 * t:128 * (t + 1)],
                                        xhat[:, t, :], ident[:])
                    if t == 3:
                        nc.scalar.activation(xt_flat[:, 0:512],
                                             ptx[:, 0:512], AF.Copy)
                nc.vector.tensor_copy(out=xt_flat[:, 512:1024],
                                      in_=ptx[:, 512:1024])

            # ---- local y norms (for the diagonal only)
            for t in range(TX):
                nc.vector.scalar_tensor_tensor(
                    out=scrD.tile([128, 128], f32, tag='sq', name='sq')[:],
                    in0=ydnat[:, t, :], scalar=1.0, in1=ydnat[:, t, :],
                    op0=ALU.mult, op1=ALU.mult,
                    accum_out=nyd2[:, t:t + 1])
            nc.vector.reciprocal(nyd2[:], nyd2[:])
            nc.scalar.sqrt(rnyd[:], nyd2[:])

            # ---- diagonal dots (raw x.y per local row) + scale
            for t in range(TX):
                nc.vector.scalar_tensor_tensor(
                    out=scrD.tile([128, 128], f32, tag='dg', name='dg')[:],
                    in0=xnat[:, t, :], scalar=1.0, in1=ydnat[:, t, :],
                    op0=ALU.mult, op1=ALU.mult, accum_out=d2[:, t:t + 1])
            nc.vector.tensor_mul(t1x[:], d2[:], rnx[:])
            nc.vector.tensor_mul(sim_d[:], t1x[:], rnyd[:])

            # ---- main: per j-block f32r matmuls + fused relu-accumulate
            # ACT tiles accum into PSUM Rp, DVE tiles into SBUF R.
            # DVE also interleaves the 4 segmented ny^2 reduces.
            ndve = 0
            red_after = {12: 0, 14: 1, 16: 2, 18: 3}
            with tc.tile_pool(name="mpsum", bufs=3, space="PSUM") as mpsum:
                with tc.tile_pool(name="rpsum", bufs=1, space="PSUM") as rp:
                    Rp = rp.tile([128, TY], f32, tag="racc")
                    nc.vector.memset(Rp[:], 0.0)
                    rhs = xhatT[:].rearrange("p a b -> p (a b)")
                    for t in range(TY):
                        ps = mpsum.tile([128, 1024], f32, tag="mp")
                        lhsT = yT32[:, t, :]
                        nc.tensor.matmul(ps[:, 0:512], lhsT, rhs[:, 0:512])
                        nc.tensor.matmul(ps[:, 512:1024], lhsT,
                                         rhs[:, 512:1024])
                        if _reduce_kind(t) == "act":
                            nc.scalar.activation(
                                ps[:], ps[:], AF.Relu,
                                accum_out=Rp[:, t:t + 1])
                        else:
                            nc.vector.tensor_scalar(
                                out=ps[:], in0=ps[:], scalar1=0.0,
                                scalar2=None, op0=ALU.max, op1=ALU.add,
                                accum_out=R[:, t:t + 1])
                            ndve += 1
                            if ndve == 2:
                                # diagonal correction, off the critical path
                                nc.vector.scalar_tensor_tensor(
                                    out=scrD.tile([128, TX], f32, tag='df',
                                                  name='df')[:],
                                    in0=sim_d[:], scalar=1.0, in1=relu_d[:],
                                    op0=ALU.mult, op1=ALU.add,
                                    accum_out=outsb[:, 1:2])
                            g = red_after.get(ndve)
                            if g is not None:
                                nc.vector.tensor_reduce(
                                    out=ny2[:, RT * g:RT * (g + 1)],
                                    in_=sqn[:, RT * g:RT * (g + 1), :],
                                    op=ALU.add, axis=AX.X)
                            if ndve == 21:
                                nc.vector.reciprocal(rny[:], ny2[:])
                                nc.scalar.sqrt(rny[:], rny[:])
                        if t == 4:
                            nc.scalar.activation(relu_d[:], sim_d[:],
                                                 AF.Relu)
                    nc.vector.tensor_add(R[:], R[:], Rp[:])

            # ---- final: scale per-block sums by 1/||y_j|| and total
            nc.vector.scalar_tensor_tensor(
                out=scrD.tile([128, TY], f32, tag='fs', name='fs')[:],
                in0=R[:], scalar=1.0, in1=rny[:],
                op0=ALU.mult, op1=ALU.mult, accum_out=outsb[:, 0:1])
            nc.sync.dma_start(out=out_d[:], in_=outsb[:])

    nc.compile()
    _CACHE["nc"] = nc
    return nc


def _pmaj(a, nt):
    # host-side pure layout permute: output row (nt*p + r) holds input row
    # (128*r + p), so a contiguous-per-partition DMA lands in standard
    # tile-major SBUF layout with 128 descriptors
    n, d = a.shape
    return np.ascontiguousarray(
        a.reshape(nt, 128, d).transpose(1, 0, 2).reshape(n, d))


def _in_maps(x, y):
    yt = np.ascontiguousarray(y.T)
    yn = _pmaj(y, TY)
    maps = []
    for c in range(NCORES):
        sl = slice(SH * c, SH * (c + 1))
        maps.append({"xs": _pmaj(x[sl], TX),
                     "yd": _pmaj(y[sl], TX),
                     "yt": yt,
                     "yn": yn})
    return maps


def _combine(results):
    total = 0.0
    for c in range(NCORES):
        o = results[c]["out"].astype(np.float64)
        total += o[:, 0].sum() - o[:, 1].sum() + SH
    return np.float32(total / (float(N) * float(N)))


def _run(x, y, trace=False):
    nc = _build()
    res = run_bass_kernel_spmd(nc, _in_maps(x, y), list(range(NCORES)),
                               trace=trace)
    return _combine(res.results), res


def kernel(x, y):
    x = np.asarray(x, dtype=np.float32)
    y = np.asarray(y, dtype=np.float32)
    loss, _ = _run(x, y, trace=False)
    return loss
